# revision 65
# baseline (speedup 1.0000x reference)
"""GQA kernel for trn2: 8 NeuronCores, SPMD (b in {0,1} x 4 head-groups).

Per core (b, hg): 8 q-heads (8hg..8hg+7), 2 kv-heads (2hg, 2hg+1).
c-major software pipeline: per 512-wide q chunk c emit
  proj(c) -> RoPE -> V-build -> attention(c) with outproj(c-1) interleaved
so PE never sees a phase barrier (keeps HAM un-throttled).
V blocks are [v0|ones64|v1] so the attn@V matmul also produces the softmax
denominator replicated across 64 partitions (normalize = recip + 2 muls,
no partition broadcast).

v2: all inputs host-relaid so every DMA is one flat [128, N] descriptor
(sync engine issues descriptors at ~650ns each -> keep the count tiny);
bf16 x/W; PE warm-up matmuls at t=0 so HAM is at K=8/8 when real work
lands; chunk-0 projection is kd-outer pairwise so each xt sub-DMA unlocks
dense PE work; PSUM evacuations moved off the ACT engine (exp cadence);
fp16 output partials summed on host (row-parallel Wo all-reduce).
"""
import numpy as np
import ml_dtypes
import concourse.bass as bass
import concourse.mybir as mybir
from concourse import tile, bacc
from concourse.bass_utils import run_bass_kernel_spmd

B, S, D = 2, 2048, 2048
H, KVH, DH = 32, 8, 64
SCALE = DH ** -0.5
SC = 4          # Sq chunks of 512
KD = 16         # D contraction chunks of 128
NJ = 16         # Sk blocks of 128
F32 = mybir.dt.float32
BF16 = mybir.dt.bfloat16
F16 = mybir.dt.float16

_cache = {}


def build():
    nc = bacc.Bacc('TRN2', target_bir_lowering=False, debug=False, num_devices=8)
    # host-relaid flat layouts: one [128, N] contiguous DMA per logical load
    xT_p = nc.declare_dram_parameter('xTr', [128, SC * KD * 512], BF16, isOutput=False)
    WTkv_p = nc.declare_dram_parameter('WTkv', [128, KD * 256], BF16, isOutput=False)
    WTq_p = nc.declare_dram_parameter('WTq', [128, KD * 512], BF16, isOutput=False)
    WoT_p = nc.declare_dram_parameter('WoT', [128, 4 * D], F16, isOutput=False)
    cos4_p = nc.declare_dram_parameter('cos4', [128, S], F16, isOutput=False)
    sin4_p = nc.declare_dram_parameter('sin4', [128, S], F16, isOutput=False)
    mask_p = nc.declare_dram_parameter('mask', [128, 4 * 512], BF16, isOutput=False)
    ident_p = nc.declare_dram_parameter('ident', [128, 128], F32, isOutput=False)
    out_p = nc.declare_dram_parameter('out', [S, D], F16, isOutput=True)

    with tile.TileContext(nc) as tc:
        with tc.tile_pool(name='w', bufs=1) as wpool, \
             tc.tile_pool(name='x', bufs=2) as xpool, \
             tc.tile_pool(name='q', bufs=8) as qpool, \
             tc.tile_pool(name='ao', bufs=8) as aopool, \
             tc.tile_pool(name='v', bufs=2) as vpool, \
             tc.tile_pool(name='t', bufs=2) as tpool, \
             tc.tile_pool(name='at', bufs=4) as atpool, \
             tc.tile_pool(name='s', bufs=2) as spool, \
             tc.tile_pool(name='o', bufs=2) as opool, \
             tc.tile_pool(name='acc', bufs=2, space='PSUM') as accp, \
             tc.tile_pool(name='sc', bufs=2, space='PSUM') as scp, \
             tc.tile_pool(name='po', bufs=1, space='PSUM') as pop:

            WTkv = wpool.tile([128, KD * 256], BF16, tag='WTkv')
            WTq = wpool.tile([128, KD * 512], BF16, tag='WTq')
            cos4 = wpool.tile([128, S], F16, tag='cos4')
            sin4 = wpool.tile([128, S], F16, tag='sin4')
            masks = wpool.tile([128, 4 * 512], BF16, tag='masks')
            ident = wpool.tile([128, 128], F32, tag='ident')
            k4 = wpool.tile([128, S], BF16, tag='k4')
            kswap = wpool.tile([128, S], BF16, tag='kswap')
            V = wpool.tile([128, NJ * 256], BF16, tag='V')
            WoT = wpool.tile([128, 4 * D], F16, tag='WoT')
            scr = wpool.tile([128, 512], BF16, tag='scr')

            # PE warm-up: dummy matmuls on a zero tile keep the HAM activity
            # window busy from right after engine boot so the first real proj
            # matmuls run at K=8/8 (2.4 GHz) instead of cold 1.2 GHz.
            # (memset on gpsimd: it boots early and has nothing else to do)
            nc.gpsimd.memset(scr[:], 0.0)
            for w in range(28):
                wps = scp.tile([128, 1024], F32, tag='sc', name=f'wps{w}')
                nc.tensor.matmul(wps[:, 0:256], scr[:, 0:128], scr[:, 0:256],
                                 start=True, stop=True)

            # startup DMAs, ordered so the consumer of each lands just-in-time
            # (sync engine issues one descriptor per ~650ns -- order matters)
            xt0 = xpool.tile([128, KD * 512], BF16, tag='xt', name='xt0')
            nc.sync.dma_start(out=WTkv[:, 0:256], in_=WTkv_p[:, 0:256])
            nc.sync.dma_start(out=xt0[:, 0:512], in_=xT_p[:, 0:512])
            nc.sync.dma_start(out=WTkv[:, 256:1024], in_=WTkv_p[:, 256:1024])
            nc.sync.dma_start(out=xt0[:, 512:2048], in_=xT_p[:, 512:2048])
            nc.sync.dma_start(out=WTq[:], in_=WTq_p[:])
            nc.sync.dma_start(out=xt0[:, 2048:4096], in_=xT_p[:, 2048:4096])
            nc.sync.dma_start(out=WTkv[:, 1024:2048], in_=WTkv_p[:, 1024:2048])
            nc.sync.dma_start(out=xt0[:, 4096:6144], in_=xT_p[:, 4096:6144])
            nc.sync.dma_start(out=WTkv[:, 2048:3072], in_=WTkv_p[:, 2048:3072])
            nc.sync.dma_start(out=xt0[:, 6144:8192], in_=xT_p[:, 6144:8192])
            nc.sync.dma_start(out=WTkv[:, 3072:4096], in_=WTkv_p[:, 3072:4096])
            nc.sync.dma_start(out=cos4[:], in_=cos4_p[:])
            nc.sync.dma_start(out=sin4[:], in_=sin4_p[:])
            nc.sync.dma_start(out=ident[:], in_=ident_p[:])
            nc.sync.dma_start(out=masks[:], in_=mask_p[:])
            nc.vector.memset(V[:], 1.0)
            # zero-fill the attention-weight slots once: subrange exps leave
            # masked column ranges unwritten, and first-rotation SBUF could
            # hold NaN bit patterns that would survive the x0 mask multiply
            for z in range(4):
                atz = atpool.tile([128, 1024], BF16, tag='at', name=f'atz{z}')
                nc.gpsimd.memset(atz[:], 0.0)

            aout_c = {}   # (c, hp) -> per-chunk fp16 attention-output tile
            ost_c = {}    # (cc, sb) -> shared per-sb output staging tile

            def outproj_unit(cc, sb, dg):
                # one (s-block, 1024-wide D group): hc-major so the aout block
                # stays the PE stationary across each dc pair (ldweights reuse)
                if (cc, sb) not in ost_c:
                    ost_c[(cc, sb)] = opool.tile([128, 2048], F16, tag='ost',
                                                 name=f'ost{sb}')
                ost = ost_c[(cc, sb)]
                dc0, dc1 = 2 * dg, 2 * dg + 1
                po0 = accp.tile([128, 512], F32, tag='acc', name=f'po{sb}_{dc0}')
                po1 = accp.tile([128, 512], F32, tag='acc', name=f'po{sb}_{dc1}')
                for hc in range(4):
                    lhs = aout_c[(cc, hc)][:, 128 * (sb - 4 * cc):128 * (sb - 4 * cc) + 128]
                    nc.tensor.matmul(po0[:], lhs, WoT[:, hc * D + 512 * dc0: hc * D + 512 * (dc0 + 1)],
                                     start=(hc == 0), stop=(hc == 3))
                    nc.tensor.matmul(po1[:], lhs, WoT[:, hc * D + 512 * dc1: hc * D + 512 * (dc1 + 1)],
                                     start=(hc == 0), stop=(hc == 3))
                # ACT is exp-bound in the late chunks; route the evacuations
                # to DVE there (DVE has slack once proj fillers taper off)
                if cc >= 2:
                    nc.vector.tensor_copy(ost[:, 1024 * dg:1024 * dg + 512], po0[:])
                else:
                    nc.scalar.copy(ost[:, 1024 * dg:1024 * dg + 512], po0[:])
                nc.vector.tensor_copy(ost[:, 1024 * dg + 512:1024 * (dg + 1)], po1[:])
                if dg == 1:
                    nc.sync.dma_start(out=out_p[128 * sb:128 * (sb + 1), :],
                                      in_=ost[:])

            qc_c = {}    # chunk -> list of 4 q tiles
            xts_c = {0: xt0}

            def emit_xt_dmas(cc):
                if cc in xts_c or cc >= SC:
                    return
                xt = xpool.tile([128, KD * 512], BF16, tag='xt', name=f'xt{cc}')
                nc.sync.dma_start(out=xt[:],
                                  in_=xT_p[:, 8192 * cc:8192 * (cc + 1)])
                xts_c[cc] = xt

            def wslice(m, kd):
                if m < 4:
                    return WTq[:, 512 * kd + 128 * m: 512 * kd + 128 * (m + 1)]
                return WTkv[:, 256 * kd + 128 * (m - 4): 256 * kd + 128 * (m - 3)]

            ps_open = {}

            def vbuild(cc, vT):
                # V natural (bf16): per j [v0|ones|v1|ones] (4x64).
                # Chunks 1-3 build via the DMA transpose XBAR (no PE
                # transposes, no PSUM bounce, no DVE casts) -- their vbuilds
                # are fillers with long lead time. Chunk 0 has no lead time
                # before attention(0), so it keeps the fast PE path.
                for j in range(4 * cc, 4 * cc + 4):
                    jj = j - 4 * cc
                    if cc <= 1:
                        pt = accp.tile([128, 128], F32, tag='acc', name=f'pt{j}')
                        nc.tensor.transpose(pt[:], vT[:, 128 * jj:128 * jj + 128],
                                            ident[:])
                        nc.vector.tensor_copy(V[:, 256 * j:256 * j + 64], pt[:, 0:64])
                        nc.vector.tensor_copy(V[:, 256 * j + 128:256 * j + 192],
                                              pt[:, 64:128])
                    else:
                        nc.sync.dma_start_transpose(
                            out=V[:, 256 * j:256 * j + 64],
                            in_=vT[0:64, 128 * jj:128 * jj + 128])
                        nc.sync.dma_start_transpose(
                            out=V[:, 256 * j + 128:256 * j + 192],
                            in_=vT[64:128, 128 * jj:128 * jj + 128])

            vT_c = {}

            def proj_epilogue(cc, m, ps):
                cs = slice(512 * cc, 512 * (cc + 1))
                if m < 5:
                    # RoPE: out = ps*cos4 + swap32(ps)*sin4 (sign baked in
                    # sin4). The partition-crossed muls must read PSUM (the
                    # BIR verifier requires equal base partitions when both
                    # inputs are SBUF); ps readers are emitted first so the
                    # accumulator slot frees as early as possible.
                    t1 = tpool.tile([128, 512], F32, tag='t1')
                    t2 = tpool.tile([128, 512], F32, tag='t2')
                    for g in range(2):
                        b0 = 64 * g
                        nc.vector.tensor_mul(t2[b0:b0 + 32, :], ps[b0 + 32:b0 + 64, :],
                                             sin4[b0:b0 + 32, cs])
                        nc.vector.tensor_mul(t2[b0 + 32:b0 + 64, :], ps[b0:b0 + 32, :],
                                             sin4[b0 + 32:b0 + 64, cs])
                    nc.vector.tensor_mul(t1[:], ps[:], cos4[:, cs])
                    tgt = qc_c[cc][m][:] if m < 4 else k4[:, cs]
                    nc.vector.tensor_add(tgt, t1[:], t2[:])
                    if m == 4:
                        # kT replication: rep0=[kv0|kv0], rep1=[kv1|kv1]
                        nc.vector.tensor_add(kswap[0:64, cs], t1[64:128, :], t2[64:128, :])
                        nc.vector.tensor_add(kswap[64:128, cs], t1[0:64, :], t2[0:64, :])
                else:
                    vT = vpool.tile([128, 512], F32 if cc <= 1 else BF16, tag='vT')
                    nc.scalar.copy(vT[:], ps[:])
                    vT_c[cc] = vT

            def vbuild_unit(cc):
                vbuild(cc, vT_c.pop(cc))

            def proj_unit(cc, m, half=None):
                # one projection output block (128 rows) + its epilogue;
                # half=0/1 emits only that kd half (finer filler granularity)
                xt = xts_c[cc]
                if half == 0:
                    ps = accp.tile([128, 512], F32, tag='acc', name=f'ps{cc}_{m}')
                    ps_open[(cc, m)] = ps
                    for kd in range(8):
                        nc.tensor.matmul(ps[:], wslice(m, kd),
                                         xt[:, 512 * kd:512 * (kd + 1)],
                                         start=(kd == 0), stop=False)
                    return
                if half == 1:
                    ps = ps_open.pop((cc, m))
                    for kd in range(8, KD):
                        nc.tensor.matmul(ps[:], wslice(m, kd),
                                         xt[:, 512 * kd:512 * (kd + 1)],
                                         start=False, stop=(kd == KD - 1))
                else:
                    ps = accp.tile([128, 512], F32, tag='acc', name=f'ps{cc}_{m}')
                    for kd in range(KD):
                        nc.tensor.matmul(ps[:], wslice(m, kd),
                                         xt[:, 512 * kd:512 * (kd + 1)],
                                         start=(kd == 0), stop=(kd == KD - 1))
                proj_epilogue(cc, m, ps)

            # chunk 0: pairwise kd-outer so each 2048-wide xt sub-DMA unlocks
            # a dense run of matmuls; accumulators live in the (still idle)
            # scores-psum pool so the next pair's matmuls never wait on the
            # previous pair's RoPE drain (accp stays free for V transposes)
            qc_c[0] = [qpool.tile([128, 512], BF16, tag='qc', name=f'qc0_{m}')
                       for m in range(4)]
            emit_xt_dmas(1)

            def dummy_mms(n, tag_n):
                # HAM-keepalive: the chunk-0 phase is DMA-paced; these keep
                # the PE activity window busy through the data-wait holes so
                # the real matmuls run at 2.4 GHz.
                dps = accp.tile([128, 512], F32, tag='acc', name=f'dm{tag_n}')
                for w in range(n):
                    nc.tensor.matmul(dps[:, 0:256], scr[:, 0:128], scr[:, 0:256],
                                     start=True, stop=True)

            def c0_pair(ma, mb, fill):
                pspair = scp.tile([128, 1024], F32, tag='sc', name=f'ps0_{ma}{mb}')
                psa, psb = pspair[:, 0:512], pspair[:, 512:1024]
                for kd in range(KD):
                    nc.tensor.matmul(psa, wslice(ma, kd),
                                     xt0[:, 512 * kd:512 * (kd + 1)],
                                     start=(kd == 0), stop=(kd == KD - 1))
                    nc.tensor.matmul(psb, wslice(mb, kd),
                                     xt0[:, 512 * kd:512 * (kd + 1)],
                                     start=(kd == 0), stop=(kd == KD - 1))
                    if fill and kd in (3, 7, 11):
                        dummy_mms(3, f'{ma}_{kd}')
                return psa, psb

            # chunk-0 emission order is tuned around the strict per-engine
            # FIFOs: the PE queue must never hold an instruction whose deps
            # aren't met when it reaches the head. vT evacuation (ACT) is
            # emitted right after the (4,5) matmuls; the V transposes (PE)
            # only after pair (0,1)'s matmuls so vT is ready by then.
            ps4, ps5 = c0_pair(4, 5, True)
            proj_epilogue(0, 5, ps5)   # vT evacuation (ACT)
            proj_epilogue(0, 4, ps4)   # RoPE k (DVE)
            dummy_mms(3, 'p01')
            psa, psb = c0_pair(0, 1, True)
            vbuild_unit(0)
            proj_epilogue(0, 0, psa)
            proj_epilogue(0, 1, psb)
            psa, psb = c0_pair(2, 3, False)
            proj_epilogue(0, 2, psa)
            proj_epilogue(0, 3, psb)

            nc.sync.dma_start(out=WoT[:], in_=WoT_p[:])
            # k/v projection of chunk 1 right away: PE meat while the DVE
            # drains chunk-0 RoPE and attention(0)'s latency chains run
            qc_c[1] = [qpool.tile([128, 512], BF16, tag='qc', name=f'qc1_{m}')
                       for m in range(4)]
            proj_unit(1, 4)
            proj_unit(1, 5)

            for c in range(SC):
                # filler units spread through attention(c):
                # outproj of chunk c-1, then projection blocks of chunk c+1
                fillers = []
                if c == 0:
                    # chunk-1 k/v matmuls ran pre-attention; V transposes of
                    # chunk 1 go here so they don't block the PE FIFO
                    fillers.append((vbuild_unit, (1,)))
                if c >= 1:
                    for sb in range(4 * (c - 1), 4 * c):
                        for dg in range(2):
                            fillers.append((outproj_unit, (c - 1, sb, dg)))
                if c + 1 < SC:
                    emit_xt_dmas(c + 1)
                    if c + 1 >= 2:
                        qc_c[c + 1] = [qpool.tile([128, 512], BF16, tag='qc',
                                                  name=f'qc{c+1}_{m}')
                                       for m in range(4)]
                    ms = (0, 1, 2, 3) if c == 0 else (4, 5, 0, 1, 2, 3)
                    for m in ms:
                        fillers.append((proj_unit, (c + 1, m, 0)))
                        fillers.append((proj_unit, (c + 1, m, 1)))
                        if m == 5:
                            fillers.append((vbuild_unit, (c + 1,)))
                nj = 4 * c + 4
                # diag j's (the last 4 of each hp) carry the long
                # exp->mask->attnv latency chain: weight them double so
                # fillers land there and keep the PE fed
                npoints = 4 * (nj - 1 + 4)
                nfil = len(fillers)
                thresholds = [round((f + 1) * npoints / (nfil + 1)) for f in range(nfil)]
                point = 0
                qc = qc_c[c]

                for hp in range(4):
                    kv = hp // 2
                    oAB = pop.tile([128, 1024], F32, tag='oab')
                    atas = {}

                    def scores(j):
                        sct = scp.tile([128, 1024], F32, tag='sc', name=f'sc{hp}_{c}_{j}')
                        if kv == 0:
                            kA = k4[0:64, 128 * j:128 * (j + 1)]
                            kB = kswap[64:128, 128 * j:128 * (j + 1)]
                        else:
                            kA = kswap[0:64, 128 * j:128 * (j + 1)]
                            kB = k4[64:128, 128 * j:128 * (j + 1)]
                        nc.tensor.matmul(sct[:, 0:512], kA, qc[hp][0:64, :], start=True, stop=True)
                        nc.tensor.matmul(sct[:, 512:1024], kB, qc[hp][64:128, :], start=True, stop=True)
                        ata = atpool.tile([128, 1024], BF16, tag='at', name=f'at{hp}_{c}_{j}')
                        d = j - 4 * c
                        if 1 <= d <= 3:
                            # diagonal block: columns [0,128d) are entirely
                            # above the causal boundary -- skip them in the
                            # exp (the mask multiply below zeroes them; the
                            # skipped region holds finite stale data)
                            atar = ata[:].rearrange("p (h q) -> p h q", h=2)[:, :, 128 * d:]
                            sctr = sct[:].rearrange("p (h q) -> p h q", h=2)[:, :, 128 * d:]
                            nc.scalar.activation(atar, sctr,
                                                 mybir.ActivationFunctionType.Exp,
                                                 scale=SCALE)
                        else:
                            nc.scalar.activation(ata[:], sct[:],
                                                 mybir.ActivationFunctionType.Exp,
                                                 scale=SCALE)
                        if 0 <= d <= 3:
                            nc.vector.tensor_mul(ata[:, 0:512], ata[:, 0:512],
                                                 masks[:, 512 * d:512 * (d + 1)])
                            nc.gpsimd.tensor_mul(ata[:, 512:1024], ata[:, 512:1024],
                                                 masks[:, 512 * d:512 * (d + 1)])
                        atas[j] = ata

                    def attnv(j):
                        # both heads share the kv head -> identical [v|ones]
                        # stationary for A and B (rows 0:64 data, 64:128 denom)
                        ata = atas.pop(j)
                        vs = V[:, 256 * j + 128 * kv:256 * j + 128 * kv + 128]
                        nc.tensor.matmul(oAB[:, 0:512], vs,
                                         ata[:, 0:512], start=(j == 0), stop=(j == nj - 1))
                        nc.tensor.matmul(oAB[:, 512:1024], vs,
                                         ata[:, 512:1024], start=(j == 0), stop=(j == nj - 1))

                    def keepalive():
                        # zero-adding accumulate into the live oAB bank:
                        # numerically a no-op (scr is zeros), but keeps the
                        # PE activity window busy through the exp/mask
                        # latency chains so HAM stays at K=8/8
                        nc.tensor.matmul(oAB[:, 0:256], scr[:, 0:128],
                                         scr[:, 0:256], start=False, stop=False,
                                         skip_group_check=True)

                    scores(0)
                    for j in range(1, nj):
                        scores(j)
                        # keepalive BEFORE attnv: when attnv blocks the PE
                        # FIFO head waiting on exp/mask, anything emitted
                        # after it can't run -- the zero-add must precede it
                        if c == 0 or (c == 1 and j >= nj - 4):
                            keepalive()
                        attnv(j - 1)
                        point += 2 if j >= nj - 4 else 1
                        while fillers and thresholds and point >= thresholds[0]:
                            thresholds.pop(0)
                            fn, args = fillers.pop(0)
                            fn(*args)
                    attnv(nj - 1)
                    # hp boundary: next hp's first attnv waits on this hp's
                    # oAB drain (single psum buffer) -- keep the PE FIFO fed
                    for _ in range(2):
                        if fillers:
                            if thresholds:
                                thresholds.pop(0)
                            fn, args = fillers.pop(0)
                            fn(*args)

                    # normalize: evacuate PSUM into base-0 data/denom tiles,
                    # approx recip on the denom tile, two base-aligned muls.
                    # denom copy goes to DVE in the late (exp-bound) chunks
                    # normalize: evacuate PSUM into base-0 data/denom tiles on
                    # parallel engines (oAB frees after both), recip + muls.
                    # The denom copy moves to DVE in the exp-bound late chunks.
                    dsw = spool.tile([64, 1024], F32, tag='dsw')
                    sbD = spool.tile([64, 1024], F32, tag='sbd')
                    if c >= 2:
                        # keep ACT exp-only here: both copies on DVE
                        nc.vector.tensor_copy(dsw[:], oAB[64:128, :])
                        nc.vector.tensor_copy(sbD[:], oAB[0:64, :])
                    else:
                        nc.scalar.copy(dsw[:], oAB[64:128, :])
                        nc.vector.tensor_copy(sbD[:], oAB[0:64, :])
                    rr = spool.tile([64, 1024], F32, tag='rr')
                    nc.vector.reciprocal_approx_fast(rr[:], dsw[:])
                    ao = aopool.tile([128, 512], F16, tag='ao', name=f'ao{c}_{hp}')
                    aout_c[(c, hp)] = ao
                    nc.vector.tensor_mul(ao[0:64, :], sbD[:, 0:512], rr[:, 0:512])
                    nc.vector.tensor_mul(ao[64:128, :], sbD[:, 512:1024], rr[:, 512:1024])

                for fn, args in fillers:
                    fn(*args)

            # bridge the final normalize chain (~5us of ACT/DVE with no PE
            # work) so HAM stays warm and the tail outproj runs at 2.4 GHz
            dummy_mms(80, 'tail')
            for sb in range(12, 16):
                for dg in range(2):
                    outproj_unit(3, sb, dg)
    nc.compile()
    return nc


_PERM = np.concatenate([np.arange(0, DH, 2), np.arange(1, DH, 2)])


def _prep_core(x, Wq, Wk, Wv, Wo, cos, sin, b, hg):
    # q heads 8hg..8hg+7 permuted, kv heads 2hg,2hg+1 (k permuted, v natural)
    wq = Wq.reshape(H, DH, D)[8 * hg:8 * hg + 8][:, _PERM, :].reshape(512, D)
    wk = Wk.reshape(KVH, DH, D)[2 * hg:2 * hg + 2][:, _PERM, :].reshape(128, D)
    wv = Wv.reshape(KVH, DH, D)[2 * hg:2 * hg + 2].reshape(128, D)
    # host relayout -> flat [128, N] kd-major images (single-descriptor DMAs)
    xT = x[b].T.astype(ml_dtypes.bfloat16)                         # (D, S)
    xTr = np.ascontiguousarray(
        xT.reshape(KD, 128, SC, 512).transpose(1, 2, 0, 3).reshape(128, SC * KD * 512))
    wkv = np.concatenate([wk, wv], 0).T.astype(ml_dtypes.bfloat16)  # (D, 256)
    WTkv = np.ascontiguousarray(
        wkv.reshape(KD, 128, 256).transpose(1, 0, 2).reshape(128, KD * 256))
    wqT = wq.T.astype(ml_dtypes.bfloat16)                           # (D, 512)
    WTq = np.ascontiguousarray(
        wqT.reshape(KD, 128, 512).transpose(1, 0, 2).reshape(128, KD * 512))
    WoTn = Wo[:, 512 * hg:512 * (hg + 1)].T.astype(np.float16)      # (512, D)
    WoT = np.ascontiguousarray(
        WoTn.reshape(4, 128, D).transpose(1, 0, 2).reshape(128, 4 * D))
    cosT = np.ascontiguousarray(cos.T.astype(np.float32))           # (32, S)
    sinT = np.ascontiguousarray(sin.T.astype(np.float32))
    cos4 = np.tile(cosT, (4, 1)).astype(np.float16)
    sin4 = np.concatenate([-sinT, sinT, -sinT, sinT], 0).astype(np.float16)
    mask = np.zeros((128, 4 * 512), dtype=np.float64)
    for dd in range(4):
        blk = (128 * dd + np.arange(128)[:, None]) <= np.arange(512)[None, :]
        mask[:, 512 * dd:512 * (dd + 1)] = blk
    return {'xTr': xTr, 'WTkv': WTkv, 'WTq': WTq, 'WoT': WoT,
            'cos4': cos4, 'sin4': sin4,
            'mask': mask.astype(ml_dtypes.bfloat16),
            'ident': np.eye(128, dtype=np.float32)}


def _run(inputs, trace=False, tmpdir=None):
    if 'nc' not in _cache:
        _cache['nc'] = build()
    in_maps = [_prep_core(inputs['x'], inputs['Wq'], inputs['Wk'], inputs['Wv'],
                          inputs['Wo'], inputs['cos'], inputs['sin'], c // 4, c % 4)
               for c in range(8)]
    res = run_bass_kernel_spmd(_cache['nc'], in_maps, core_ids=list(range(8)),
                               trace=trace, tmpdir=tmpdir)
    parts = [res.results[c]['out'].astype(np.float32) for c in range(8)]
    out = np.stack([parts[0] + parts[1] + parts[2] + parts[3],
                    parts[4] + parts[5] + parts[6] + parts[7]], 0)
    return out, res


def kernel(**inputs):
    out, _ = _run(inputs, trace=False)
    return out
